# revision 1
# baseline (speedup 1.0000x reference)
"""CrissCross (axial) attention kernel for 8 TRN2 NeuronCores.

Shapes (hardcoded): x [16, 512, 64, 64], Wq/Wk [64, 512], Wv [512, 512].
Sharding: data-parallel over batch, 2 batches per core.

Per-core, per-batch pipeline:
  1. qk = [Wq;Wk] @ x  (fp32r matmuls, bias fused into psum->sbuf copy, bf16)
  2. energies per column w / per row h, bf16, two orientations at once
     (orientation-2 [g,h]/[u,w] -> exp -> P tiles, the aggregation lhsT;
      orientation-1 [h,g]/[w,u] -> exp -> free-axis reduce -> softmax sums).
     No max subtraction (|e| << 80 so fp32 exp is safe). The reference's -inf
     diag mask becomes zeroing the diag of the column part post-exp.
  3. R = 1/(Sc+Sr) assembled via tiny DMAs + PE transposes in both packings.
  4. vT = (gamma*Wv @ x)^T in both spatial orders, via projection matmuls
     whose lhsT M-dim is a (strided) spatial slice of x. fp32r, bf16 out.
  5. aggregation out[h,c] = P^T @ vT per column/row, 2-way packed on the PE
     via partition halves; 1/S fused into the psum->sbuf copy as a
     per-partition scale.
  6. PE-transpose the aggregated parts to channel-major, accumulating both
     parts directly into one t buffer (column part written through a strided
     AP that undoes the (w,h) ordering; row part added on top).
  7. out = t + x via gpsimd, DMA out.
"""

import json
import sys

import ml_dtypes
import numpy as np

sys.path.insert(0, "/root/.axon_site")

from contextlib import ExitStack

import concourse.bass as bass
import concourse.bass2jax as b2j
import concourse.mybir as mybir
import concourse.tile as tile
from concourse.bass_utils import run_bass_kernel_spmd

F32 = mybir.dt.float32
F32R = mybir.dt.float32r
BF16 = mybir.dt.bfloat16
AF = mybir.ActivationFunctionType
NE = mybir.AluOpType.not_equal

B, C, H, W = 16, 512, 64, 64
S = H * W            # 4096
NB = 2               # batches per core
NCORES = 8

_PATCHED = False


def _install_multiwait_split():
    """This container's walrus rejects instructions carrying >1 sem waits.
    Split extras into standalone EventSemaphore waits on the same engine,
    inserted immediately before (preserves per-engine program order)."""
    global _PATCHED
    if _PATCHED:
        return
    _PATCHED = True
    orig = b2j._decompress_ant_bir

    def _split(s):
        d = json.loads(orig(s))
        for fn in d.get("functions", []):
            for blk in fn.get("blocks", []):
                out = []
                for ins in blk.get("instructions", []):
                    si = ins.get("sync_info")
                    ow = (si or {}).get("on_wait") or []
                    if len(ow) > 1:
                        for i, w in enumerate(ow[:-1]):
                            out.append({
                                "debug": ins.get("debug", 0),
                                "engine": ins["engine"],
                                "ins": [], "outs": [],
                                "name": f'{ins["name"]}-xw{i}',
                                "opcode": "EventSemaphore",
                                "sync_info": {"on_update": [], "on_wait": [w]},
                            })
                        si["on_wait"] = [ow[-1]]
                    out.append(ins)
                blk["instructions"] = out
        return json.dumps(d).encode()

    b2j._decompress_ant_bir = _split


def build_nc():
    nc = bass.Bass("TRN2", target_bir_lowering=False, debug=False)

    x_d = nc.dram_tensor("x", [NB, C, S], F32R, kind="ExternalInput").ap()
    wqk_d = nc.dram_tensor("wqkT", [C, 128], F32R, kind="ExternalInput").ap()
    bqk_d = nc.dram_tensor("bqk", [128, 1], F32, kind="ExternalInput").ap()
    wv_d = nc.dram_tensor("wvT", [C, C], F32R, kind="ExternalInput").ap()
    gbv_d = nc.dram_tensor("gbv", [C, 1], F32, kind="ExternalInput").ap()
    idf_d = nc.dram_tensor("identf", [128, 128], F32, kind="ExternalInput").ap()
    idb_d = nc.dram_tensor("identb", [128, 128], BF16, kind="ExternalInput").ap()
    out_d = nc.dram_tensor("out", [NB, C, S], F32, kind="ExternalOutput").ap()

    x_v = x_d.rearrange("b (kc p) s -> b p kc s", p=128)
    out_v = out_d.rearrange("b (kc p) s -> b p kc s", p=128)
    wqk_v = wqk_d.rearrange("(kc p) m -> p kc m", p=128)
    wv_v = wv_d.rearrange("(kc p) m -> p kc m", p=128)
    gbv_v = gbv_d.rearrange("(kc p) one -> p kc one", p=128)

    with tile.TileContext(nc) as tc, ExitStack() as ctx:
        consts = ctx.enter_context(tc.tile_pool(name="consts", bufs=1))
        wqk_sb = consts.tile([128, 4, 128], F32R)
        nc.sync.dma_start(out=wqk_sb, in_=wqk_v)
        wv_sb = consts.tile([128, 4, 512], F32R)
        nc.sync.dma_start(out=wv_sb, in_=wv_v)
        bqk_sb = consts.tile([128, 1], F32)
        nc.sync.dma_start(out=bqk_sb, in_=bqk_d)
        gbv_sb = consts.tile([128, 4, 1], F32)
        nc.sync.dma_start(out=gbv_sb, in_=gbv_v)
        idf_sb = consts.tile([128, 128], F32)
        nc.sync.dma_start(out=idf_sb, in_=idf_d)
        idb_sb = consts.tile([128, 128], BF16)
        nc.sync.dma_start(out=idb_sb, in_=idb_d)

        # psum pools (8 banks total)
        pp_proj = ctx.enter_context(tc.tile_pool(name="pp_proj", bufs=2, space="PSUM"))
        pp_en = ctx.enter_context(tc.tile_pool(name="pp_en", bufs=1, space="PSUM"))
        pp_agg = ctx.enter_context(tc.tile_pool(name="pp_agg", bufs=2, space="PSUM"))
        pp_tr = ctx.enter_context(tc.tile_pool(name="pp_tr", bufs=1, space="PSUM"))
        pp_sm = ctx.enter_context(tc.tile_pool(name="pp_sm", bufs=1, space="PSUM"))

        px = ctx.enter_context(tc.tile_pool(name="px", bufs=1))
        pqk = ctx.enter_context(tc.tile_pool(name="pqk", bufs=1))
        pP = ctx.enter_context(tc.tile_pool(name="pP", bufs=1))
        pscr = ctx.enter_context(tc.tile_pool(name="pscr", bufs=1))
        pst = ctx.enter_context(tc.tile_pool(name="pst", bufs=2))
        pvt = ctx.enter_context(tc.tile_pool(name="pvt", bufs=1))
        put = ctx.enter_context(tc.tile_pool(name="put", bufs=1))
        pt = ctx.enter_context(tc.tile_pool(name="pt", bufs=1))
        po = ctx.enter_context(tc.tile_pool(name="po", bufs=1))

        for b in range(NB):
            # ---- load x ----------------------------------------------------
            x_sb = px.tile([128, 4, S], F32R, tag="x")
            for kc in range(4):
                nc.sync.dma_start(out=x_sb[:, kc, :], in_=x_v[b, :, kc, :])

            # ---- qk projection --------------------------------------------
            qkA = po.tile([128, S], BF16, tag="o")
            qkB = pt.tile([128, S], BF16, tag="t")
            for n in range(8):
                ps = pp_proj.tile([128, 512], F32, tag="proj")
                for kc in range(4):
                    nc.tensor.matmul(
                        ps,
                        lhsT=wqk_sb[:, kc, :],
                        rhs=x_sb[:, kc, n * 512:(n + 1) * 512],
                        start=(kc == 0), stop=(kc == 3),
                    )
                nc.scalar.activation(
                    out=qkA[:, n * 512:(n + 1) * 512], in_=ps,
                    func=AF.Identity, bias=bqk_sb, scale=1.0,
                )
            # B = partition-swapped copy of A (k on top, q on bottom)
            nc.sync.dma_start(out=qkB[0:64, :], in_=qkA[64:128, :])
            nc.sync.dma_start(out=qkB[64:128, :], in_=qkA[0:64, :])

            Acol = qkA.rearrange("p (h w) -> p w h", w=W)
            Bcol = qkB.rearrange("p (h w) -> p w h", w=W)
            Arow = qkA.rearrange("p (h w) -> p h w", h=H)
            Brow = qkB.rearrange("p (h w) -> p h w", h=H)

            # ---- energies + softmax sums ----------------------------------
            PcolT = pP.tile([128, 2048], BF16, tag="PcolT")
            ProwT = pP.tile([128, 2048], BF16, tag="ProwT")
            Sc_p = pst.tile([128, 32], F32, tag="Scp")
            Sr_p = pst.tile([128, 32], F32, tag="Srp")

            for part in ("col", "row"):
                P_sb = PcolT if part == "col" else ProwT
                S_sb = Sc_p if part == "col" else Sr_p
                Ksrc = Bcol if part == "col" else Brow   # k in top half
                Qsrc = Acol if part == "col" else Arow   # q in top half
                for bi in range(4):
                    o2 = pp_en.tile([128, 512], F32, tag="o2")
                    o1 = pp_en.tile([128, 512], F32, tag="o1")
                    for sl in range(8):
                        m = bi * 8 + sl
                        w0, w1 = 2 * m, 2 * m + 1
                        fs = slice(sl * 64, (sl + 1) * 64)
                        # orientation-2: out [g, h] (resp. [u, w])
                        nc.tensor.matmul(o2[0:64, fs], lhsT=Ksrc[0:64, w0, :],
                                         rhs=Qsrc[0:64, w0, :], start=True, stop=True)
                        nc.tensor.matmul(o2[64:128, fs], lhsT=Qsrc[64:128, w1, :],
                                         rhs=Ksrc[64:128, w1, :], start=True, stop=True)
                        # orientation-1: out [h, g] (resp. [w, u])
                        nc.tensor.matmul(o1[0:64, fs], lhsT=Qsrc[0:64, w0, :],
                                         rhs=Ksrc[0:64, w0, :], start=True, stop=True)
                        nc.tensor.matmul(o1[64:128, fs], lhsT=Ksrc[64:128, w1, :],
                                         rhs=Qsrc[64:128, w1, :], start=True, stop=True)
                    bs = slice(bi * 512, (bi + 1) * 512)
                    nc.scalar.activation(out=P_sb[:, bs], in_=o2, func=AF.Exp)
                    scr = pscr.tile([128, 512], F32, tag="scr")
                    nc.scalar.activation(out=scr, in_=o1, func=AF.Exp)
                    if part == "col":
                        scr3 = scr.rearrange("p (m g) -> p m g", g=64)
                        nc.gpsimd.affine_select(
                            out=scr3[0:64], in_=scr3[0:64],
                            pattern=[[0, 8], [-1, 64]], compare_op=NE,
                            fill=0.0, base=0, channel_multiplier=1)
                        nc.gpsimd.affine_select(
                            out=scr3[64:128], in_=scr3[64:128],
                            pattern=[[0, 8], [-1, 64]], compare_op=NE,
                            fill=0.0, base=0, channel_multiplier=1)
                    nc.vector.reduce_sum(
                        out=S_sb[:, bi * 8:(bi + 1) * 8],
                        in_=scr.rearrange("p (m g) -> p m g", g=64),
                        axis=mybir.AxisListType.X)
                if part == "col":
                    P3 = P_sb.rearrange("p (m h) -> p m h", h=64)
                    nc.gpsimd.affine_select(
                        out=P3[0:64], in_=P3[0:64],
                        pattern=[[0, 32], [-1, 64]], compare_op=NE,
                        fill=0.0, base=0, channel_multiplier=1)
                    nc.gpsimd.affine_select(
                        out=P3[64:128], in_=P3[64:128],
                        pattern=[[0, 32], [-1, 64]], compare_op=NE,
                        fill=0.0, base=0, channel_multiplier=1)


            # ---- per spatial order: project vT, aggregate, transpose ------
            # t accumulates the channel-major attention output (bf16).
            t_sb = pt.tile([128, 4, S], BF16, tag="t")
            xw = x_sb.rearrange("p kc (h w) -> p kc w h", w=W)
            for part in ("col", "row"):
                vt = pvt.tile([128, 32, 512], BF16, tag="vt")
                for j in range(32):
                    ps = pp_proj.tile([128, 512], F32, tag="proj")
                    if part == "col":
                        # walrus rejects dst-partition-64 matmuls with full-K
                        # rows; compute both halves at base 0 and DMA-shift
                        # the odd half into vt[64:128].
                        ps2 = pp_en.tile([64, 512], F32, tag="o2")
                        for kc in range(4):
                            nc.tensor.matmul(
                                ps[0:64, :], lhsT=xw[:, kc, 2 * j, :],
                                rhs=wv_sb[:, kc, :],
                                start=(kc == 0), stop=(kc == 3))
                            nc.tensor.matmul(
                                ps2, lhsT=xw[:, kc, 2 * j + 1, :],
                                rhs=wv_sb[:, kc, :],
                                start=(kc == 0), stop=(kc == 3))
                        stg = pst.tile([64, 512], BF16, tag="stg")
                        nc.vector.tensor_copy(stg, ps2)
                        nc.gpsimd.dma_start(out=vt[64:128, j, :], in_=stg)
                        sl_copy = vt[0:64, j, :]
                        if j % 2 == 0:
                            nc.vector.tensor_copy(sl_copy, ps[0:64, :])
                        else:
                            nc.scalar.activation(out=sl_copy, in_=ps[0:64, :],
                                                 func=AF.Identity)
                        continue
                    if True:
                        for kc in range(4):
                            nc.tensor.matmul(
                                ps, lhsT=x_sb[:, kc, j * 128:(j + 1) * 128],
                                rhs=wv_sb[:, kc, :],
                                start=(kc == 0), stop=(kc == 3))
                    if j % 2 == 0:
                        nc.scalar.activation(out=vt[:, j, :], in_=ps, func=AF.Identity)
                    else:
                        nc.vector.tensor_copy(vt[:, j, :], ps)

                if part == "col":
                    # ---- stats: R = 1/(Sc + Sr) in both pack layouts ---------------
                    Sc_pl = pst.tile([64, 64], F32, tag="Scpl")   # [h, w]
                    Sr_pl = pst.tile([64, 64], F32, tag="Srpl")   # [w, h]
                    Sc2 = Sc_pl.rearrange("p (m t) -> p m t", t=2)
                    Sr2 = Sr_pl.rearrange("p (m t) -> p m t", t=2)
                    nc.sync.dma_start(out=Sc2[:, :, 0], in_=Sc_p[0:64, :])
                    nc.sync.dma_start(out=Sc2[:, :, 1], in_=Sc_p[64:128, :])
                    nc.sync.dma_start(out=Sr2[:, :, 0], in_=Sr_p[0:64, :])
                    nc.sync.dma_start(out=Sr2[:, :, 1], in_=Sr_p[64:128, :])
                    tp = pp_sm.tile([64, 64], F32, tag="stps")
                    nc.tensor.transpose(tp, Sr_pl, idf_sb[0:64, 0:64])   # -> [h, w]
                    R_hw = pst.tile([64, 64], F32, tag="Rhw")
                    nc.vector.tensor_add(R_hw, Sc_pl, tp)
                    nc.vector.reciprocal(out=R_hw, in_=R_hw)
                    tp2 = pp_sm.tile([64, 64], F32, tag="stps")
                    nc.tensor.transpose(tp2, R_hw, idf_sb[0:64, 0:64])   # -> [w, h]
                    R_wh = pst.tile([64, 64], F32, tag="Rwh")
                    nc.vector.tensor_copy(R_wh, tp2)
                    R_cs = pst.tile([128, 32], F32, tag="Rcs")
                    R_rs = pst.tile([128, 32], F32, tag="Rrs")
                    Rhw2 = R_hw.rearrange("p (m t) -> p m t", t=2)
                    Rwh2 = R_wh.rearrange("p (m t) -> p m t", t=2)
                    nc.sync.dma_start(out=R_cs[0:64, :], in_=Rhw2[:, :, 0])
                    nc.sync.dma_start(out=R_cs[64:128, :], in_=Rhw2[:, :, 1])
                    nc.sync.dma_start(out=R_rs[0:64, :], in_=Rwh2[:, :, 0])
                    nc.sync.dma_start(out=R_rs[64:128, :], in_=Rwh2[:, :, 1])

                P_sb = PcolT if part == "col" else ProwT
                R_sb = R_cs if part == "col" else R_rs
                ut = put.tile([128, 32, 512], BF16, tag="ut")
                for m in range(32):
                    aps = pp_agg.tile([128, 512], F32, tag="agg")
                    nc.tensor.matmul(aps[0:64, :],
                                     lhsT=P_sb[0:64, m * 64:(m + 1) * 64],
                                     rhs=vt[0:64, m, :], start=True, stop=True)
                    nc.tensor.matmul(aps[64:128, :],
                                     lhsT=P_sb[64:128, m * 64:(m + 1) * 64],
                                     rhs=vt[64:128, m, :], start=True, stop=True)
                    if m % 2 == 0:
                        nc.scalar.activation(out=ut[:, m, :], in_=aps,
                                             func=AF.Copy, scale=R_sb[:, m:m + 1])
                    else:
                        nc.vector.tensor_scalar_mul(out=ut[:, m, :], in0=aps,
                                                    scalar1=R_sb[:, m:m + 1])

                # PE-transpose to channel-major, accumulate into t
                for cc in range(4):
                    for g4 in range(8):
                        tps = pp_tr.tile([128, 512], BF16, tag="tr")
                        for mm in range(4):
                            m = g4 * 4 + mm
                            nc.tensor.transpose(
                                tps[:, mm * 128:(mm + 1) * 128],
                                ut[:, m, cc * 128:(cc + 1) * 128], idb_sb)
                        if part == "col":
                            # write with reorder: (w,h)-ordered data -> (h,w)
                            # src block mm holds s2 in [m*128,(m+1)*128):
                            # col j2 = h + 64*tpar, w = 2m + tpar
                            src = tps.rearrange("p (mm tpar h) -> p mm tpar h",
                                                mm=4, tpar=2)
                            dst = t_sb[:, cc, :].rearrange(
                                "p (h wq mm tpar) -> p mm tpar h wq",
                                h=64, wq=8, mm=4)[:, :, :, :, g4]
                            nc.scalar.activation(out=dst, in_=src, func=AF.Identity,
                                                 bias=gbv_sb[:, cc, :])
                        else:
                            # row part: contiguous slice, add on top
                            dst = t_sb[:, cc, g4 * 512:(g4 + 1) * 512]
                            nc.vector.tensor_add(dst, tps, dst)

            # ---- final: out = t + x ---------------------------------------
            for cc in range(4):
                for hh in range(2):
                    fs = slice(hh * 2048, (hh + 1) * 2048)
                    o_sb = po.tile([128, 2048], F32, tag="o")
                    nc.gpsimd.tensor_add(o_sb, t_sb[:, cc, fs],
                                         x_sb[:, cc, fs].bitcast(F32))
                    nc.sync.dma_start(out=out_v[b, :, cc, fs], in_=o_sb)

    return nc


_NC = None


def kernel(x, Wq, bq, Wk, bk, Wv, bv, gamma):
    global _NC
    _install_multiwait_split()
    x = np.ascontiguousarray(np.asarray(x, dtype=np.float32))
    Wq = np.asarray(Wq, np.float32); Wk = np.asarray(Wk, np.float32)
    Wv = np.asarray(Wv, np.float32)
    bq = np.asarray(bq, np.float32); bk = np.asarray(bk, np.float32)
    bv = np.asarray(bv, np.float32); gamma = np.asarray(gamma, np.float32)

    wqkT = np.ascontiguousarray(np.concatenate([Wq, Wk], 0).T)       # [512, 128]
    bqk = np.ascontiguousarray(np.concatenate([bq, bk])[:, None])    # [128, 1]
    wvT = np.ascontiguousarray((gamma[0] * Wv).T)                    # [512, 512]
    gbv = np.ascontiguousarray((gamma[0] * bv)[:, None])             # [512, 1]
    identf = np.eye(128, dtype=np.float32)
    identb = np.eye(128, dtype=np.float32).astype(ml_dtypes.bfloat16)

    if _NC is None:
        _NC = build_nc()

    xs = x.reshape(B, C, S)
    in_maps = []
    for i in range(NCORES):
        in_maps.append({
            "x": np.ascontiguousarray(xs[i * NB:(i + 1) * NB]),
            "wqkT": wqkT, "bqk": bqk, "wvT": wvT, "gbv": gbv,
            "identf": identf, "identb": identb,
        })
    res = run_bass_kernel_spmd(_NC, in_maps, list(range(NCORES)))
    out = np.empty((B, C, S), np.float32)
    for i in range(NCORES):
        out[i * NB:(i + 1) * NB] = res.results[i]["out"]
    return out.reshape(B, C, H, W)



# revision 15
# speedup vs baseline: 1.1569x; 1.1569x over previous
"""CrissCross (axial) attention kernel for 8 TRN2 NeuronCores.

Shapes (hardcoded): x [16, 512, 64, 64], Wq/Wk [64, 512], Wv [512, 512].
Sharding: data-parallel over batch, 2 batches per core.

Per-core, per-batch pipeline (v2 — direct channel-major aggregation):
  1. qk = [Wq;Wk] @ x  (f32r matmuls, bias fused into psum->sbuf copy, bf16)
  2. energies per column w / row h in two orientations:
     o2 [g,h]/[u,w] -> exp -> P tiles (agg rhs); o1 [h,g]/[w,u] -> exp ->
     free-axis reduce -> softmax sums.  Diag of the column part zeroed
     post-exp (gpsimd affine_select).  No max subtraction (|e| small).
  3. R = 1/(Sc+Sr) assembled [h,w] and [w,h]; gathered to single-partition
     rows; broadcast to all partitions via a K=2 matmul with a half-select
     stationary; P *= Rv on DVE (P becomes the normalized attention, bf16).
  4. vT = (gamma*Wv @ x)^T in both spatial orders ((h,w) and (w,h)) via
     f32r matmuls with x-slices as lhsT (M=128 even for the (w,h) order
     using a 2D-free lhsT AP); psum->sbuf bf16 copies on scalar.
  5. Aggregation directly in channel-major: out[c,h]|w = vt^T @ P per
     spatial line, N=64 matmuls, accumulated per 8-line groups in psum.
  6. Column part drained as T1 = x + (Ucol+gbv) in-place over x (DVE
     scalar_tensor_tensor, strided (w,h)->(h,w) dst); row part finished as
     out = Urow + T1 into bf16 staging, DMA'd out per channel chunk.
"""

import json
import os
import sys

import ml_dtypes
import numpy as np

sys.path.insert(0, "/root/.axon_site")

from contextlib import ExitStack

import concourse.bass as bass
import concourse.bass2jax as b2j
import concourse.mybir as mybir
import concourse.tile as tile
from concourse.bass_utils import run_bass_kernel_spmd

F32 = mybir.dt.float32
F32R = mybir.dt.float32r
BF16 = mybir.dt.bfloat16
AF = mybir.ActivationFunctionType
ALU = mybir.AluOpType
NE = mybir.AluOpType.not_equal

STAGE = int(os.environ.get("KSTAGE", "9"))
B, C, H, W = 16, 512, 64, 64
S = H * W            # 4096
NB = 2               # batches per core
NCORES = 8

_PATCHED = False


def _install_multiwait_split():
    """This container's walrus rejects instructions carrying >1 sem waits.
    Split extras into standalone EventSemaphore waits on the same engine,
    inserted immediately before (preserves per-engine program order)."""
    global _PATCHED
    if _PATCHED:
        return
    _PATCHED = True
    orig = b2j._decompress_ant_bir

    def _split(s):
        d = json.loads(orig(s))
        for fn in d.get("functions", []):
            for blk in fn.get("blocks", []):
                out = []
                for ins in blk.get("instructions", []):
                    si = ins.get("sync_info")
                    ow = (si or {}).get("on_wait") or []
                    if len(ow) > 1:
                        for i, w in enumerate(ow[:-1]):
                            out.append({
                                "debug": ins.get("debug", 0),
                                "engine": ins["engine"],
                                "ins": [], "outs": [],
                                "name": f'{ins["name"]}-xw{i}',
                                "opcode": "EventSemaphore",
                                "sync_info": {"on_update": [], "on_wait": [w]},
                            })
                        si["on_wait"] = [ow[-1]]
                    out.append(ins)
                blk["instructions"] = out
        return json.dumps(d).encode()

    b2j._decompress_ant_bir = _split


def build_nc():
    nc = bass.Bass("TRN2", target_bir_lowering=False, debug=False)

    x_d = nc.dram_tensor("x", [NB, C, S], F32R, kind="ExternalInput").ap()
    wqk_d = nc.dram_tensor("wqkT", [C, 128], F32R, kind="ExternalInput").ap()
    bqk_d = nc.dram_tensor("bqk", [128, 1], F32, kind="ExternalInput").ap()
    wv_d = nc.dram_tensor("wvT", [C, C], F32R, kind="ExternalInput").ap()
    gbv_d = nc.dram_tensor("gbv", [C, 1], F32, kind="ExternalInput").ap()
    idf_d = nc.dram_tensor("identf", [128, 128], F32, kind="ExternalInput").ap()
    esel_d = nc.dram_tensor("esel", [64, 32 * 128], BF16, kind="ExternalInput").ap()
    out_d = nc.dram_tensor("out", [NB, C, S], BF16, kind="ExternalOutput").ap()

    x_v = x_d.rearrange("b (kc p) s -> b p kc s", p=128)
    out_v = out_d.rearrange("b (kc p) s -> b p kc s", p=128)
    wqk_v = wqk_d.rearrange("(kc p) m -> p kc m", p=128)
    wv_v = wv_d.rearrange("(kc p) m -> p kc m", p=128)
    gbv_v = gbv_d.rearrange("(kc p) one -> p kc one", p=128)

    with tile.TileContext(nc) as tc, ExitStack() as ctx:
        consts = ctx.enter_context(tc.tile_pool(name="consts", bufs=1))
        wqk_sb = consts.tile([128, 4, 128], F32R)
        nc.sync.dma_start(out=wqk_sb, in_=wqk_v)
        wv_sb = consts.tile([128, 4, 512], F32R)
        nc.sync.dma_start(out=wv_sb, in_=wv_v)
        bqk_sb = consts.tile([128, 1], F32)
        nc.sync.dma_start(out=bqk_sb, in_=bqk_d)
        gbv_sb = consts.tile([128, 4, 1], F32)
        nc.sync.dma_start(out=gbv_sb, in_=gbv_v)
        idf_sb = consts.tile([128, 128], F32)
        nc.sync.dma_start(out=idf_sb, in_=idf_d)
        esel_sb = consts.tile([64, 32, 128], BF16)
        nc.sync.dma_start(out=esel_sb, in_=esel_d.rearrange("w (m p) -> w m p", p=128))

        # psum pools (8 banks total): proj 2 + en/agg 4 + rv/stats 2
        pp_proj = ctx.enter_context(tc.tile_pool(name="pp_proj", bufs=2, space="PSUM"))
        pp_ea = ctx.enter_context(tc.tile_pool(name="pp_ea", bufs=2, space="PSUM"))
        pp_rv = ctx.enter_context(tc.tile_pool(name="pp_rv", bufs=1, space="PSUM"))

        px = ctx.enter_context(tc.tile_pool(name="px", bufs=1))
        pqk = ctx.enter_context(tc.tile_pool(name="pqk", bufs=4))
        pP = ctx.enter_context(tc.tile_pool(name="pP", bufs=1))
        pscr = ctx.enter_context(tc.tile_pool(name="pscr", bufs=2))
        pst = ctx.enter_context(tc.tile_pool(name="pst", bufs=1))
        pvt = ctx.enter_context(tc.tile_pool(name="pvt", bufs=1))

        for b in range(NB):
            # ---- load x ----------------------------------------------------
            x_sb = px.tile([128, 4, S], F32R, tag="x")
            for kc in range(4):
                eng = (nc.sync, nc.scalar, nc.gpsimd, nc.sync)[kc]
                eng.dma_start(out=x_sb[:, kc, 0:2048], in_=x_v[b, :, kc, 0:2048])
                eng.dma_start(out=x_sb[:, kc, 2048:4096], in_=x_v[b, :, kc, 2048:4096])

            # ---- qk projection --------------------------------------------
            qkA = pqk.tile([128, S], BF16, tag="qk")
            qkB = pqk.tile([128, S], BF16, tag="qk")
            for n in range(8):
                ps = pp_proj.tile([128, 512], F32, tag="proj")
                for kc in range(4):
                    nc.tensor.matmul(
                        ps,
                        lhsT=wqk_sb[:, kc, :],
                        rhs=x_sb[:, kc, n * 512:(n + 1) * 512],
                        start=(kc == 0), stop=(kc == 3),
                    )
                nc.scalar.activation(
                    out=qkA[:, n * 512:(n + 1) * 512], in_=ps,
                    func=AF.Identity, bias=bqk_sb, scale=1.0,
                )
            # B = partition-swapped copy of A (k on top, q on bottom)
            nc.sync.dma_start(out=qkB[0:64, :], in_=qkA[64:128, :])
            nc.sync.dma_start(out=qkB[64:128, :], in_=qkA[0:64, :])

            Acol = qkA.rearrange("p (h w) -> p w h", w=W)
            Bcol = qkB.rearrange("p (h w) -> p w h", w=W)
            Arow = qkA.rearrange("p (h w) -> p h w", h=H)
            Brow = qkB.rearrange("p (h w) -> p h w", h=H)

            if STAGE < 2:
                continue
            # ---- energies + softmax sums ----------------------------------
            PcolT = pP.tile([128, 2048], BF16, tag="Pcol")
            ProwT = pP.tile([128, 2048], BF16, tag="Prow")
            Sc_p = pst.tile([128, 32], F32, tag="Scp")
            Sr_p = pst.tile([128, 32], F32, tag="Srp")

            for part in ("col", "row"):
                P_sb = PcolT if part == "col" else ProwT
                S_sb = Sc_p if part == "col" else Sr_p
                Ksrc = Bcol if part == "col" else Brow   # k in top half
                Qsrc = Acol if part == "col" else Arow   # q in top half
                for bi in range(4):
                    o2 = pp_ea.tile([128, 512], F32, tag="en")
                    o1 = pp_ea.tile([128, 512], F32, tag="en")
                    for sl in range(8):
                        m = bi * 8 + sl
                        w0, w1 = 2 * m, 2 * m + 1
                        fs = slice(sl * 64, (sl + 1) * 64)
                        # orientation-2: out [g, h] (resp. [u, w])
                        nc.tensor.matmul(o2[0:64, fs], lhsT=Ksrc[0:64, w0, :],
                                         rhs=Qsrc[0:64, w0, :], start=True, stop=True)
                        nc.tensor.matmul(o2[64:128, fs], lhsT=Qsrc[64:128, w1, :],
                                         rhs=Ksrc[64:128, w1, :], start=True, stop=True)
                        # orientation-1: out [h, g] (resp. [w, u])
                        nc.tensor.matmul(o1[0:64, fs], lhsT=Qsrc[0:64, w0, :],
                                         rhs=Ksrc[0:64, w0, :], start=True, stop=True)
                        nc.tensor.matmul(o1[64:128, fs], lhsT=Ksrc[64:128, w1, :],
                                         rhs=Qsrc[64:128, w1, :], start=True, stop=True)
                    bs = slice(bi * 512, (bi + 1) * 512)
                    nc.scalar.activation(out=P_sb[:, bs], in_=o2, func=AF.Exp)
                    scr = pscr.tile([128, 512], F32, tag="scr")
                    nc.scalar.activation(out=scr, in_=o1, func=AF.Exp)
                    if part == "col":
                        scr3 = scr.rearrange("p (m g) -> p m g", g=64)
                        nc.gpsimd.affine_select(
                            out=scr3[0:64], in_=scr3[0:64],
                            pattern=[[0, 8], [-1, 64]], compare_op=NE,
                            fill=0.0, base=0, channel_multiplier=1)
                        nc.gpsimd.affine_select(
                            out=scr3[64:128], in_=scr3[64:128],
                            pattern=[[0, 8], [-1, 64]], compare_op=NE,
                            fill=0.0, base=0, channel_multiplier=1)
                    nc.vector.reduce_sum(
                        out=S_sb[:, bi * 8:(bi + 1) * 8],
                        in_=scr.rearrange("p (m g) -> p m g", g=64),
                        axis=mybir.AxisListType.X)
                if part == "col":
                    P3 = P_sb.rearrange("p (m h) -> p m h", h=64)
                    nc.gpsimd.affine_select(
                        out=P3[0:64], in_=P3[0:64],
                        pattern=[[0, 32], [-1, 64]], compare_op=NE,
                        fill=0.0, base=0, channel_multiplier=1)
                    nc.gpsimd.affine_select(
                        out=P3[64:128], in_=P3[64:128],
                        pattern=[[0, 32], [-1, 64]], compare_op=NE,
                        fill=0.0, base=0, channel_multiplier=1)

            if STAGE < 3:
                continue
            # ---- stats: R = 1/(Sc + Sr) in [h,w] and [w,h] layouts --------
            Sc_pl = pst.tile([64, 64], F32, tag="Scpl")   # [h, w]
            Sr_pl = pst.tile([64, 64], F32, tag="Srpl")   # [w, h]
            Sc2 = Sc_pl.rearrange("p (m t) -> p m t", t=2)
            Sr2 = Sr_pl.rearrange("p (m t) -> p m t", t=2)
            nc.sync.dma_start(out=Sc2[:, :, 0], in_=Sc_p[0:64, :])
            nc.sync.dma_start(out=Sc2[:, :, 1], in_=Sc_p[64:128, :])
            nc.sync.dma_start(out=Sr2[:, :, 0], in_=Sr_p[0:64, :])
            nc.sync.dma_start(out=Sr2[:, :, 1], in_=Sr_p[64:128, :])
            tpt = pp_rv.tile([128, 512], F32, tag="rv")
            tp = tpt[0:64, 0:64]
            nc.tensor.transpose(tp, Sr_pl, idf_sb[0:64, 0:64])   # -> [h, w]
            R_hw = pst.tile([64, 64], F32, tag="Rhw")
            nc.vector.tensor_add(R_hw, Sc_pl, tp)
            nc.vector.reciprocal(out=R_hw, in_=R_hw)
            tpt2 = pp_rv.tile([128, 512], F32, tag="rv")
            tp2 = tpt2[0:64, 0:64]
            nc.tensor.transpose(tp2, R_hw, idf_sb[0:64, 0:64])   # -> [w, h]
            R_wh = pst.tile([64, 64], F32, tag="Rwh")
            nc.vector.tensor_copy(R_wh, tp2)

            # bf16 copies of R for the Rv selection matmuls
            Rwh16 = pst.tile([64, 64], BF16, tag="Rwh16")
            Rhw16 = pst.tile([64, 64], BF16, tag="Rhw16")
            nc.vector.tensor_copy(Rwh16, R_wh)
            nc.vector.tensor_copy(Rhw16, R_hw)

            # broadcast Rv across partitions via selection matmuls and fold
            # into P (normalize): Rv[p, (m,h)] = R_wh[2m + (p>=64), h]
            for part in ("col", "row"):
                P_sb = PcolT if part == "col" else ProwT
                R16 = Rwh16 if part == "col" else Rhw16
                for q in range(4):
                    fs = slice(q * 512, (q + 1) * 512)
                    rv = pp_rv.tile([128, 512], F32, tag="rv")
                    for ml in range(8):
                        m = q * 8 + ml
                        nc.tensor.matmul(rv[:, ml * 64:(ml + 1) * 64],
                                         lhsT=esel_sb[:, m, :], rhs=R16,
                                         start=True, stop=True)
                    nc.vector.tensor_mul(P_sb[:, fs], P_sb[:, fs], rv)

            if STAGE < 4:
                continue

            # odd-parity P relocated to partition base 0 (matmul operands at
            # base 64 with out at base 0 crash the PE tile config)
            Pc_o = pP.tile([64, 2048], BF16, tag="Pco")
            Pr_o = pP.tile([64, 2048], BF16, tag="Pro")
            nc.sync.dma_start(out=Pc_o, in_=PcolT[64:128, :])
            nc.sync.dma_start(out=Pr_o, in_=ProwT[64:128, :])

            if STAGE < 5:
                continue

            # ---- fused v-projection + aggregation, channel-major ----------
            # col part: T1 = (Ucol + gbv) + x -> bf16 (strided (w,h) dst)
            xw = x_sb.rearrange("p kc (h w) -> p kc w h", w=W)
            xcol = xw
            T1s = []
            for cc in range(4):
                T1 = pqk.tile([128, S], BF16, tag="qk")
                T1s.append(T1)
            for oct_ in range(8):
                vt_ce = pvt.tile([64, 4, 512], BF16, tag="vtce", bufs=2)
                vt_co = pvt.tile([64, 4, 512], BF16, tag="vtco", bufs=2)
                for jj in range(4):
                    j = oct_ * 4 + jj
                    ps = pp_proj.tile([128, 512], F32, tag="proj")
                    ps2 = pp_ea.tile([128, 512], F32, tag="en")
                    for kc in range(4):
                        nc.tensor.matmul(ps[0:64, :], lhsT=xw[:, kc, 2 * j, :],
                                         rhs=wv_sb[:, kc, :],
                                         start=(kc == 0), stop=(kc == 3))
                        nc.tensor.matmul(ps2[0:64, :], lhsT=xw[:, kc, 2 * j + 1, :],
                                         rhs=wv_sb[:, kc, :],
                                         start=(kc == 0), stop=(kc == 3))
                    if jj % 2 == 0:
                        nc.scalar.activation(out=vt_ce[:, jj, :], in_=ps[0:64, :],
                                             func=AF.Identity)
                        nc.vector.tensor_copy(vt_co[:, jj, :], ps2[0:64, :])
                    else:
                        nc.vector.tensor_copy(vt_ce[:, jj, :], ps[0:64, :])
                        nc.scalar.activation(out=vt_co[:, jj, :], in_=ps2[0:64, :],
                                             func=AF.Identity)
                for cc in range(4):
                    cs = slice(cc * 128, (cc + 1) * 128)
                    aps = pp_ea.tile([128, 512], F32, tag="agg", bufs=3)
                    for wl in range(8):
                        w = oct_ * 8 + wl
                        j, half = w // 2, w % 2
                        jj = j - oct_ * 4
                        vt = vt_ce if half == 0 else vt_co
                        Prhs = PcolT[0:64, j * 64:(j + 1) * 64] if half == 0                             else Pc_o[:, j * 64:(j + 1) * 64]
                        nc.tensor.matmul(
                            aps[:, wl * 64:(wl + 1) * 64],
                            lhsT=vt[:, jj, cs], rhs=Prhs,
                            start=True, stop=True)
                    osl = slice(oct_ * 8, (oct_ + 1) * 8)
                    T1col = T1s[cc].rearrange("p (h w) -> p w h", w=W)
                    nc.vector.scalar_tensor_tensor(
                        out=T1col[:, osl, :], in0=aps, scalar=gbv_sb[:, cc, :],
                        in1=xcol[:, cc, osl, :].bitcast(F32),
                        op0=ALU.add, op1=ALU.add)

            if STAGE < 7:
                continue
            # row part: out = Urow + T1 in-place on T1, DMA out per chunk
            for oct_ in range(8):
                vt_rc = pvt.tile([128, 4, 512], BF16, tag="vtrc", bufs=2)
                vt_ro = pvt.tile([64, 4, 512], BF16, tag="vtro", bufs=2)
                for jj in range(4):
                    j = oct_ * 4 + jj
                    ps = pp_proj.tile([128, 512], F32, tag="proj")
                    for kc in range(4):
                        nc.tensor.matmul(ps, lhsT=x_sb[:, kc, j * 128:(j + 1) * 128],
                                         rhs=wv_sb[:, kc, :],
                                         start=(kc == 0), stop=(kc == 3))
                    if jj % 2 == 0:
                        nc.scalar.activation(out=vt_rc[:, jj, :], in_=ps,
                                             func=AF.Identity)
                    else:
                        nc.vector.tensor_copy(vt_rc[:, jj, :], ps)
                nc.gpsimd.dma_start(out=vt_ro, in_=vt_rc[64:128, :, :])
                for cc in range(4):
                    cs = slice(cc * 128, (cc + 1) * 128)
                    aps = pp_ea.tile([128, 512], F32, tag="agg", bufs=3)
                    for hl in range(8):
                        h = oct_ * 8 + hl
                        j, half = h // 2, h % 2
                        jj = j - oct_ * 4
                        vt = vt_rc[0:64] if half == 0 else vt_ro
                        Prhs = ProwT[0:64, j * 64:(j + 1) * 64] if half == 0                             else Pr_o[:, j * 64:(j + 1) * 64]
                        nc.tensor.matmul(
                            aps[:, hl * 64:(hl + 1) * 64],
                            lhsT=vt[:, jj, cs], rhs=Prhs,
                            start=True, stop=True)
                    fs = slice(oct_ * 512, (oct_ + 1) * 512)
                    nc.vector.tensor_add(T1s[cc][:, fs], aps, T1s[cc][:, fs])
            for cc in range(4):
                nc.sync.dma_start(out=out_v[b, :, cc, :], in_=T1s[cc])

    return nc


_NC = None


def prep_in_maps(x, Wq, bq, Wk, bk, Wv, bv, gamma):
    x = np.ascontiguousarray(np.asarray(x, dtype=np.float32))
    Wq = np.asarray(Wq, np.float32); Wk = np.asarray(Wk, np.float32)
    Wv = np.asarray(Wv, np.float32)
    bq = np.asarray(bq, np.float32); bk = np.asarray(bk, np.float32)
    bv = np.asarray(bv, np.float32); gamma = np.asarray(gamma, np.float32)

    wqkT = np.ascontiguousarray(np.concatenate([Wq, Wk], 0).T)       # [512, 128]
    bqk = np.ascontiguousarray(np.concatenate([bq, bk])[:, None])    # [128, 1]
    wvT = np.ascontiguousarray((gamma[0] * Wv).T)                    # [512, 512]
    gbv = np.ascontiguousarray((gamma[0] * bv)[:, None])             # [512, 1]
    identf = np.eye(128, dtype=np.float32)
    esel = np.zeros((64, 32, 128), dtype=np.float32)
    for m in range(32):
        esel[2 * m, m, 0:64] = 1.0
        esel[2 * m + 1, m, 64:128] = 1.0
    esel = np.ascontiguousarray(
        esel.reshape(64, 32 * 128)).astype(ml_dtypes.bfloat16)

    xs = x.reshape(B, C, S)
    in_maps = []
    for i in range(NCORES):
        in_maps.append({
            "x": np.ascontiguousarray(xs[i * NB:(i + 1) * NB]),
            "wqkT": wqkT, "bqk": bqk, "wvT": wvT, "gbv": gbv,
            "identf": identf, "esel": esel,
        })
    return in_maps


def kernel(x, Wq, bq, Wk, bk, Wv, bv, gamma):
    global _NC
    _install_multiwait_split()
    in_maps = prep_in_maps(x, Wq, bq, Wk, bk, Wv, bv, gamma)
    if _NC is None:
        _NC = build_nc()
    res = run_bass_kernel_spmd(_NC, in_maps, list(range(NCORES)))
    out = np.empty((B, C, S), np.float32)
    for i in range(NCORES):
        out[i * NB:(i + 1) * NB] = np.asarray(
            res.results[i]["out"], dtype=np.float32)
    return out.reshape(B, C, H, W)
